# revision 1
# baseline (speedup 1.0000x reference)
"""Trainium2 Bass kernel for nn_AddingToQ (GNN message passing + sinkhorn).

Self-contained: takes FULL unsharded inputs, shards 256 graph pairs across
8 NeuronCores (32 pairs / 1920 nodes / 4320 real edges per core), runs an
all-SBUF matmul-formulated GNN, gathers per-core scores to the full [256]
output.

Key restructurings (validated against the jax reference in numpy):
  * mask==0 padding edges contribute nothing to segment_sum -> skipped.
  * msg MLP first layer split: concat([h_f, h_t, e]) @ W1 =
      h_f @ W1a + h_t @ W1b + (e @ W1c + b1), last term a per-edge
      layer-invariant constant (uniform row for all-ones edge features).
  * segment-sum moved before msg_w2:  agg_D = seg(relu1) @ W2 + indeg * b2;
    agg_D @ upd_w1a folded into one precomputed M1 = W2 @ upd_w1a.
  * gathers/scatter = block-banded one-hot matmuls (pair-local edges), all
    fp32r at free-dim 256 (full PE rate).
  * node state kept feature-major (hT [128, 1920]) so every matmul contracts
    on the partition dim with zero runtime transposes.
"""
import os
import numpy as np

# problem constants
B, NQ, NC = 256, 15, 30
NPG = 2 * NC
N = B * NPG
EPP = 135                 # real (mask=1) edges per pair
E_REAL = B * EPP
E_TOT = 92160
D, H, T = 128, 256, 64
N_PROP, SK_ITERS, SK_TEMP = 5, 10, 0.1
NCORES = 8
BP = B // NCORES          # 32 pairs per core
NL = BP * NPG             # 1920
EL = BP * EPP             # 4320
NT = NL // 128            # 15 node tiles
ET = (EL + 127) // 128    # 34 edge tiles
NG = (NL + 255) // 256    # 8 node groups of 256 (last 128)

_CACHE = {}


def _geometry():
    """Static (input-independent) tile maps shared by host baking and the
    device program: which node K-tiles each edge tile gathers from, and which
    256-node groups it scatters into."""
    ktiles = []   # per edge tile: list of node K-tiles (128-wide)
    groups = []   # per edge tile: list of node groups (256-wide)
    for t in range(ET):
        e0, e1 = 128 * t, min(128 * t + 128, EL)
        p0, p1 = e0 // EPP, (e1 - 1) // EPP
        n_lo, n_hi = 60 * p0, 60 * p1 + 59
        ktiles.append(list(range(n_lo // 128, n_hi // 128 + 1)))
        groups.append(list(range(n_lo // 256, n_hi // 256 + 1)))
    # flat emit orders
    gu_index = {}   # (t, k) -> idx
    for t in range(ET):
        for k in ktiles[t]:
            gu_index[(t, k)] = len(gu_index)
    gs_index = {}   # (t, g) -> idx
    for t in range(ET):
        for g in groups[t]:
            gs_index[(t, g)] = len(gs_index)
    tiles_of_group = [[t for t in range(ET) if g in groups[t]] for g in range(NG)]
    owner = {}
    for g in range(NG):
        for t in tiles_of_group[g]:
            owner.setdefault(t, g)
    return dict(ktiles=ktiles, groups=groups, gu_index=gu_index,
                gs_index=gs_index, tiles_of_group=tiles_of_group, owner=owner)


def _host_prep(inputs):
    """Fold encoders/weights, shard by pair slabs, bake one-hot mask tensors
    in their exact SBUF layouts."""
    f32 = np.float32
    g = _geometry()
    NGU = len(g['gu_index'])
    NS = len(g['gs_index'])

    msg_w1 = np.asarray(inputs['msg_w1'], f32)
    W1a, W1b, W1c = msg_w1[0:128], msg_w1[128:256], msg_w1[256:384]
    upd_w1 = np.asarray(inputs['upd_w1'], f32)
    A1, A2 = upd_w1[0:128], upd_w1[128:256]
    msg_w2 = np.asarray(inputs['msg_w2'], f32)
    M1 = (msg_w2 @ A1).astype(f32)
    b2A1 = (np.asarray(inputs['msg_b2'], f32) @ A1).astype(f32)
    upd_b1 = np.asarray(inputs['upd_b1'], f32)
    upd_w2 = np.asarray(inputs['upd_w2'], f32)
    upd_b2 = np.asarray(inputs['upd_b2'], f32)

    nf = np.asarray(inputs['node_features'], f32)
    h0 = nf * np.asarray(inputs['enc_node_w'], f32)[0][None, :] \
        + np.asarray(inputs['enc_node_b'], f32)[None, :]
    ef = np.asarray(inputs['edge_features'], f32)
    e_enc = ef * np.asarray(inputs['enc_edge_w'], f32)[0][None, :] \
        + np.asarray(inputs['enc_edge_b'], f32)[None, :]
    C_all = (e_enc @ W1c + np.asarray(inputs['msg_b1'], f32)[None, :]).astype(f32)

    from_idx = np.asarray(inputs['from_idx']).astype(np.int64)
    to_idx = np.asarray(inputs['to_idx']).astype(np.int64)
    mask = np.asarray(inputs['mask_from_idx'], f32)
    assert np.all(mask[:E_REAL] == 1.0) and np.all(mask[E_REAL:] == 0.0), \
        "unexpected mask pattern"
    pair_of_edge = np.arange(E_REAL) // EPP
    assert np.all(from_idx[:E_REAL] // NPG == pair_of_edge)
    assert np.all(to_idx[:E_REAL] // NPG == pair_of_edge)

    c_uniform = bool(np.all(C_all[:E_REAL] == C_all[0]))

    # weights shipped in exact SBUF layouts (same for all cores)
    m1_s = np.concatenate([M1[0:128, :], M1[128:256, :]], axis=1)      # [128,512]
    wu2_s = np.concatenate([upd_w2[0:128, :], upd_w2[128:256, :]], axis=1)  # [128,256]
    biasL = np.stack([b2A1, upd_b1], axis=0)                           # [2,256]
    common = {
        'w1a': np.ascontiguousarray(W1a),
        'w1b': np.ascontiguousarray(W1b),
        'm1': np.ascontiguousarray(m1_s),
        'a2': np.ascontiguousarray(A2),
        'wu2': np.ascontiguousarray(wu2_s),
        'biasl': np.ascontiguousarray(biasL),
        'ub2row': np.ascontiguousarray(upd_b2[None, :]),               # [1,128]
        'c1row': np.ascontiguousarray(C_all[0][None, :]),              # [1,256]
        'ft1': np.ascontiguousarray(np.asarray(inputs['ft1_w'], f32)),
        'ft2': np.ascontiguousarray(np.asarray(inputs['ft2_w'], f32)),
        'ft1b': np.ascontiguousarray(np.asarray(inputs['ft1_b'], f32)[:, None]),
        'ft2b': np.ascontiguousarray(np.asarray(inputs['ft2_b'], f32)[:, None]),
    }

    in_maps = []
    for c in range(NCORES):
        n0, e0 = c * NL, c * EL
        fl = from_idx[e0:e0 + EL] - n0
        tl = to_idx[e0:e0 + EL] - n0
        assert fl.min() >= 0 and fl.max() < NL and tl.min() >= 0 and tl.max() < NL

        # pad edge arrays to ET*128 with -1 (matches nothing)
        flp = np.full(ET * 128, -1, np.int64); flp[:EL] = fl
        tlp = np.full(ET * 128, -1, np.int64); tlp[:EL] = tl

        gu = np.zeros((128, NGU * 128), f32)   # lhsT gather-U: [node, edge]
        gv = np.zeros((128, NGU * 128), f32)
        for (t, k), idx in g['gu_index'].items():
            es = flp[128 * t:128 * t + 128]
            nsv = 128 * k + np.arange(128)
            gu[:, idx * 128:(idx + 1) * 128] = (es[None, :] == nsv[:, None])
            es2 = tlp[128 * t:128 * t + 128]
            gv[:, idx * 128:(idx + 1) * 128] = (es2[None, :] == nsv[:, None])
        gs = np.zeros((128, NS * 256), f32)    # rhs scatter: [edge, node]
        for (t, gg), idx in g['gs_index'].items():
            es = tlp[128 * t:128 * t + 128]
            nsv = 256 * gg + np.arange(256)
            gs[:, idx * 256:(idx + 1) * 256] = (es[:, None] == nsv[None, :])

        indeg = np.zeros(NL, f32)
        np.add.at(indeg, tl, 1.0)
        biasN = np.stack([indeg, np.ones(NL, f32)], axis=0)            # [2,1920]

        m = {
            'ht0': np.ascontiguousarray(h0[n0:n0 + NL].T),             # [128,1920]
            'gu': gu, 'gv': gv, 'gs': gs,
            'biasn': np.ascontiguousarray(biasN),
        }
        if not c_uniform:
            cs = np.zeros((ET * 128, H), f32)
            cs[:EL] = C_all[e0:e0 + EL]
            m['cstream'] = cs.reshape(ET, 128, H)
        m.update(common)
        in_maps.append(m)
    return in_maps, c_uniform, NGU, NS


def _build(c_uniform, NGU, NS):
    """Build + schedule the Bass/Tile program (identical for all cores)."""
    from contextlib import ExitStack
    import concourse.bass as bass
    import concourse.tile as tile
    from concourse import bacc, mybir
    from concourse.masks import make_identity

    f32 = mybir.dt.float32
    f32r = mybir.dt.float32r
    AF = mybir.ActivationFunctionType
    ALU = mybir.AluOpType
    AX = mybir.AxisListType
    g = _geometry()

    nc = bacc.Bacc("TRN2", target_bir_lowering=False, debug=False)

    dram = {}
    def din(name, shape, dt_=None):
        dram[name] = nc.dram_tensor(name, list(shape), dt_ or f32,
                                    kind="ExternalInput").ap()
    din('ht0', (128, NL))
    din('gu', (128, NGU * 128), f32r); din('gv', (128, NGU * 128), f32r)
    din('gs', (128, NS * 256), f32r)
    din('biasn', (2, NL))
    din('w1a', (128, H)); din('w1b', (128, H)); din('m1', (128, 2 * H))
    din('a2', (128, H)); din('wu2', (128, H)); din('biasl', (2, H))
    din('ub2row', (1, 128)); din('c1row', (1, H), f32r)
    din('ft1', (128, T)); din('ft2', (T, T)); din('ft1b', (T, 1)); din('ft2b', (T, 1))
    if not c_uniform:
        din('cstream', (ET, 128, H))
    scores_out = nc.dram_tensor('scores', [BP], f32, kind="ExternalOutput").ap()

    def r(ap):  # dtype comes from tile labels; matmuls read it directly
        return ap

    with tile.TileContext(nc) as tc:
        persist_cm = tc.tile_pool(name="persist", bufs=1)
        persist = persist_cm.__enter__()
        ps_cm = tc.tile_pool(name="ps", bufs=8, space="PSUM")
        ps = ps_cm.__enter__()

        def load(pool, name, shape, dt_=None):
            t_ = pool.tile(list(shape), dt_ or f32, tag=name)
            nc.sync.dma_start(t_[:], dram[name][:])
            return t_

        # ---- persistent tensors (live across layer + final stages) ----
        # 32 pad cols so 32-strided per-pair views in the final stage stay
        # in-bounds for the last pair
        hA = persist.tile([128, NL + 32], f32, tag="hA")
        nc.sync.dma_start(hA[:, 0:NL], dram['ht0'][:])
        nc.vector.memset(hA[:, NL:NL + 32], 0.0)
        biasn_s = load(persist, 'biasn', (2, NL))
        w1a_s = load(persist, 'w1a', (128, H)); w1b_s = load(persist, 'w1b', (128, H))
        m1_s = load(persist, 'm1', (128, 2 * H)); a2_s = load(persist, 'a2', (128, H))
        wu2_s = load(persist, 'wu2', (128, H)); biasl_s = load(persist, 'biasl', (2, H))
        ub2_s = load(persist, 'ub2row', (1, 128))
        c1_s = load(persist, 'c1row', (1, H), f32r)
        ft1_s = load(persist, 'ft1', (128, T)); ft2_s = load(persist, 'ft2', (T, T))
        ft1b_s = load(persist, 'ft1b', (T, 1)); ft2b_s = load(persist, 'ft2b', (T, 1))
        ones_s = persist.tile([1, 512], f32, tag="ones")
        nc.vector.memset(ones_s[:], 1.0)
        ones_r = persist.tile([1, 128], f32r, tag="ones_r")
        nc.scalar.activation(ones_r[:], ones_s[:, 0:128], AF.Copy)
        ones_col = persist.tile([NC, 1], f32, tag="ones_col")
        nc.vector.memset(ones_col[:], 1.0)
        ident = persist.tile([128, 128], f32, tag="ident")
        make_identity(nc, ident[:])

        # ---- propagation-scoped pools ----
        mask_cm = tc.tile_pool(name="maskp", bufs=1)
        maskp = mask_cm.__enter__()
        uv_cm = tc.tile_pool(name="uvp", bufs=1)
        uvp = uv_cm.__enter__()
        agg_cm = tc.tile_pool(name="aggp", bufs=3)
        aggpool = agg_cm.__enter__()
        rtp_cm = tc.tile_pool(name="rtp", bufs=3)
        rtpool = rtp_cm.__enter__()
        relu_cm = tc.tile_pool(name="relu1", bufs=8)
        relu_pool = relu_cm.__enter__()
        if not c_uniform:
            c_cm = tc.tile_pool(name="cstr", bufs=3)
            cpool = c_cm.__enter__()

        gu_s = load(maskp, 'gu', (128, NGU * 128), f32r)
        gv_s = load(maskp, 'gv', (128, NGU * 128), f32r)
        gs_s = load(maskp, 'gs', (128, NS * 256), f32r)
        U_s = uvp.tile([128, NT * H], f32r, tag="U")
        V_s = uvp.tile([128, NT * H], f32r, tag="V")

        for layer in range(N_PROP):
            # --- stage A: U = h @ W1a, V = h @ W1b (feature-major h) ---
            for i in range(NT):
                hs = hA[:, 128 * i:128 * (i + 1)]
                pu = ps.tile([128, 512], f32, tag="ps")
                nc.tensor.matmul(pu[:, 0:H], lhsT=r(hs), rhs=r(w1a_s[:]),
                                 start=True, stop=True)
                nc.tensor.matmul(pu[:, H:2 * H], lhsT=r(hs), rhs=r(w1b_s[:]),
                                 start=True, stop=True)
                nc.scalar.activation(U_s[:, H * i:H * (i + 1)], pu[:, 0:H], AF.Copy)
                nc.scalar.activation(V_s[:, H * i:H * (i + 1)], pu[:, H:2 * H], AF.Copy)

            # --- stage B: per 256-node group: scatter + node update ---
            relu_tiles = {}
            for grp in range(NG):
                ncols = min(256, NL - 256 * grp)
                aggp_h0 = ps.tile([128, 512], f32, tag="ps")
                aggp_h1 = ps.tile([128, 512], f32, tag="ps")
                aggp_h = [aggp_h0, aggp_h1]
                first = [True, True]
                for t in g['tiles_of_group'][grp]:
                    if g['owner'][t] == grp:
                        # relu1[t] = relu(gather_U + gather_V + c1)
                        pp = ps.tile([128, 512], f32, tag="ps")
                        kts = g['ktiles'][t]
                        for j, k in enumerate(kts):
                            idx = g['gu_index'][(t, k)]
                            nc.tensor.matmul(
                                pp[:, 0:H],
                                lhsT=r(gu_s[:, idx * 128:(idx + 1) * 128]),
                                rhs=r(U_s[:, H * k:H * (k + 1)]),
                                start=(j == 0), stop=False)
                        for j, k in enumerate(kts):
                            idx = g['gu_index'][(t, k)]
                            nc.tensor.matmul(
                                pp[:, 0:H],
                                lhsT=r(gv_s[:, idx * 128:(idx + 1) * 128]),
                                rhs=r(V_s[:, H * k:H * (k + 1)]),
                                start=False, stop=False)
                        if c_uniform:
                            nc.tensor.matmul(pp[:, 0:H], lhsT=r(ones_r[:]),
                                             rhs=r(c1_s[:]), start=False, stop=True)
                            rt = relu_pool.tile([128, H], f32r, tag="r1")
                            nc.scalar.activation(rt[:], pp[:, 0:H], AF.Relu)
                        else:
                            ct = cpool.tile([128, H], f32, tag="c")
                            nc.sync.dma_start(ct[:], dram['cstream'][t])
                            nc.tensor.matmul(pp[:, 0:H], lhsT=r(ones_r[:, 0:1]),
                                             rhs=r(c1_s[:, 0:1]), start=False,
                                             stop=True)
                            rt = relu_pool.tile([128, H], f32r, tag="r1")
                            nc.vector.tensor_add(rt[:], ct[:], pp[:, 0:H])
                            nc.vector.tensor_relu(rt[:], rt[:])
                        relu_tiles[t] = rt
                    rt = relu_tiles[t]
                    sidx = g['gs_index'][(t, grp)]
                    for hh in range(2):
                        nc.tensor.matmul(
                            aggp_h[hh][:, 0:ncols],
                            lhsT=r(rt[:, 128 * hh:128 * (hh + 1)]),
                            rhs=r(gs_s[:, sidx * 256:sidx * 256 + ncols]),
                            start=first[hh],
                            stop=(t == g['tiles_of_group'][grp][-1]))
                        first[hh] = False
                agg_g = aggpool.tile([128, 512], f32, tag="agg")
                for hh in range(2):
                    nc.scalar.activation(agg_g[:, 256 * hh:256 * hh + ncols],
                                         aggp_h[hh][:, 0:ncols], AF.Copy)

                # pre_updT + relu -> r_g  (per output-feature half)
                nslc = slice(256 * grp, 256 * grp + ncols)
                r_g = rtpool.tile([128, 512], f32, tag="rg")
                for hh in range(2):
                    pq = ps.tile([128, 512], f32, tag="ps")
                    nc.tensor.matmul(pq[:, 0:ncols],
                                     lhsT=r(m1_s[:, 128 * hh:128 * hh + 128]),
                                     rhs=r(agg_g[:, 0:ncols]),
                                     start=True, stop=False)
                    nc.tensor.matmul(pq[:, 0:ncols],
                                     lhsT=r(m1_s[:, H + 128 * hh:H + 128 * hh + 128]),
                                     rhs=r(agg_g[:, 256:256 + ncols]),
                                     start=False, stop=False)
                    nc.tensor.matmul(pq[:, 0:ncols],
                                     lhsT=r(a2_s[:, 128 * hh:128 * hh + 128]),
                                     rhs=r(hA[:, nslc]),
                                     start=False, stop=False)
                    nc.tensor.matmul(pq[:, 0:ncols],
                                     lhsT=r(biasl_s[:, 128 * hh:128 * hh + 128]),
                                     rhs=r(biasn_s[:, nslc]),
                                     start=False, stop=True)
                    nc.scalar.activation(r_g[:, 256 * hh:256 * hh + ncols],
                                         pq[:, 0:ncols], AF.Relu)
                # deltaT ; h += delta (in place)
                pd = ps.tile([128, 512], f32, tag="ps")
                nc.tensor.matmul(pd[:, 0:ncols], lhsT=r(wu2_s[:, 0:128]),
                                 rhs=r(r_g[:, 0:ncols]), start=True, stop=False)
                nc.tensor.matmul(pd[:, 0:ncols], lhsT=r(wu2_s[:, 128:256]),
                                 rhs=r(r_g[:, 256:256 + ncols]),
                                 start=False, stop=False)
                nc.tensor.matmul(pd[:, 0:ncols], lhsT=r(ub2_s[:]),
                                 rhs=r(ones_s[:, 0:ncols]), start=False, stop=True)
                nc.vector.tensor_add(hA[:, nslc], hA[:, nslc], pd[:, 0:ncols])

        # close propagation pools, freeing SBUF for the final stage
        if not c_uniform:
            c_cm.__exit__(None, None, None)
        relu_cm.__exit__(None, None, None)
        rtp_cm.__exit__(None, None, None)
        agg_cm.__exit__(None, None, None)
        uv_cm.__exit__(None, None, None)
        mask_cm.__exit__(None, None, None)

        fin_cm = tc.tile_pool(name="fin", bufs=1)
        fin = fin_cm.__enter__()
        work_cm = tc.tile_pool(name="work", bufs=4)
        work = work_cm.__enter__()

        # ---- final stage ----
        # transform all nodes: s1 = relu(ft1^T @ h + b1); tT = ft2^T @ s1 + b2
        s1_s = fin.tile([T, NL], f32, tag="s1")
        tT_s = fin.tile([T, NL], f32, tag="tT")
        for j in range(4):
            cs = slice(480 * j, 480 * (j + 1))
            p1 = ps.tile([128, 512], f32, tag="ps")
            nc.tensor.matmul(p1[0:T, 0:480], lhsT=r(ft1_s[:]), rhs=r(hA[:, cs]),
                             start=True, stop=True)
            nc.scalar.activation(s1_s[:, cs], p1[0:T, 0:480], AF.Relu, bias=ft1b_s[:])
            p2 = ps.tile([128, 512], f32, tag="ps")
            nc.tensor.matmul(p2[0:T, 0:480], lhsT=r(ft2_s[:]), rhs=r(s1_s[:, cs]),
                             start=True, stop=True)
            nc.scalar.activation(tT_s[:, cs], p2[0:T, 0:480], AF.Identity,
                                 bias=ft2b_s[:])

        # masked query transform columns: mtq [T, BP*NC], zeros at q>=NQ
        mtq_s = fin.tile([T, BP * NC], f32, tag="mtq")
        nc.vector.memset(mtq_s[:], 0.0)
        src = tT_s[:].rearrange("p (b n) -> p b n", n=NPG)[:, :, 0:NQ]
        dst = mtq_s[:].rearrange("p (b n) -> p b n", n=NC)[:, :, 0:NQ]
        nc.vector.tensor_copy(dst, src)

        # log_alpha [30, BP*30], scaled by 1/SK_TEMP
        LA_s = fin.tile([NC, BP * NC], f32, tag="LA")
        for half in range(2):
            pl = ps.tile([128, 512], f32, tag="ps")
            for pi in range(16):
                p = 16 * half + pi
                nc.tensor.matmul(
                    pl[0:NC, 30 * pi:30 * pi + 30],
                    lhsT=r(mtq_s[:, NC * p:NC * p + NC]),
                    rhs=r(tT_s[:, NPG * p + NC:NPG * p + 2 * NC]),
                    start=True, stop=True)
            nc.scalar.activation(LA_s[:, 480 * half:480 * half + 480],
                                 pl[0:NC, 0:480], AF.Copy, scale=1.0 / SK_TEMP)

        # sinkhorn
        Ebuf = fin.tile([NC, BP * NC], f32, tag="E")
        mx_s = fin.tile([NC, BP], f32, tag="mx")
        sm_s = fin.tile([NC, BP], f32, tag="sm")
        lsq_s = fin.tile([1, BP * NC], f32, tag="lsq")
        LA3 = LA_s[:].rearrange("p (b c) -> p b c", c=NC)
        mxb = mx_s[:, :, None].broadcast_to([NC, BP, NC])
        smb = sm_s[:, :, None].broadcast_to([NC, BP, NC])
        for it in range(SK_ITERS):
            # axis-2 (c within pair): lse over innermost 30
            nc.vector.tensor_reduce(mx_s[:], LA3, axis=AX.X, op=ALU.max)
            nc.vector.tensor_tensor(LA3, LA3, mxb, op=ALU.subtract)
            nc.scalar.activation(Ebuf[:], LA_s[:], AF.Exp)
            E3 = Ebuf[:].rearrange("p (b c) -> p b c", c=NC)
            nc.vector.tensor_reduce(sm_s[:], E3, axis=AX.X, op=ALU.add)
            nc.scalar.activation(sm_s[:], sm_s[:], AF.Ln)
            nc.vector.tensor_tensor(LA3, LA3, smb, op=ALU.subtract)
            # axis-1 (q across partitions): colsum of exp via ones-matmul
            nc.scalar.activation(Ebuf[:], LA_s[:], AF.Exp)
            for half in range(2):
                hs = slice(480 * half, 480 * (half + 1))
                pc = ps.tile([128, 512], f32, tag="ps")
                nc.tensor.matmul(pc[0:1, 0:480], lhsT=r(ones_col[:]),
                                 rhs=r(Ebuf[:, hs]), start=True, stop=True)
                nc.scalar.activation(lsq_s[:, hs], pc[0:1, 0:480], AF.Ln)
                pb = ps.tile([128, 512], f32, tag="ps")
                nc.tensor.matmul(pb[0:NC, 0:480], lhsT=r(ones_s[:, 0:NC]),
                                 rhs=r(lsq_s[:, hs]), start=True, stop=True)
                nc.vector.tensor_sub(LA_s[:, hs], LA_s[:, hs], pb[0:NC, 0:480])
        nc.scalar.activation(Ebuf[:], LA_s[:], AF.Exp)   # transport plan

        # batched transposes (4 pairs each, packed at stride 32 on the
        # partition axis so matmul base-partition lands on 0/32/64/96)
        tp32_s = fin.tile([NC, 32 * BP], f32, tag="tp32")
        nc.vector.memset(tp32_s[:], 0.0)
        nc.vector.tensor_copy(
            tp32_s[:].rearrange("p (b n) -> p b n", n=32)[:, :, 0:NC],
            Ebuf[:].rearrange("p (b c) -> p b c", c=NC))
        tpT_s = fin.tile([128, 8 * NC], f32, tag="tpT")
        cnm_s = fin.tile([128, 8 * D], f32, tag="cnm")
        qnm_s = fin.tile([128, 8 * D], f32, tag="qnm")

        def win32(ap_tile, off):
            """[128, 4 pairs, 32 cols] view at stride NPG starting at off."""
            w = ap_tile[:, off:off + 240]
            return w.rearrange("p (b n) -> p b n", n=NPG)[:, :, 0:32]

        for b4 in range(8):   # 4 pairs per transpose
            pt = ps.tile([128, 512], f32, tag="ps")
            tp_in = tp32_s[:, 128 * b4:128 * (b4 + 1)]
            nc.tensor.transpose(pt[0:128, 0:NC], tp_in, ident[0:NC, 0:NC])
            nc.scalar.activation(tpT_s[:, NC * b4:NC * (b4 + 1)], pt[0:128, 0:NC],
                                 AF.Copy)
            stg_c = work.tile([128, 128], f32, tag="stg")
            nc.vector.tensor_copy(
                stg_c[:].rearrange("p (b n) -> p b n", n=32),
                win32(hA, 240 * b4 + NC))
            pc_ = ps.tile([128, 512], f32, tag="ps")
            nc.tensor.transpose(pc_[0:128, 0:128], stg_c[:], ident[:])
            nc.scalar.activation(cnm_s[:, D * b4:D * (b4 + 1)], pc_[0:128, 0:128],
                                 AF.Copy)
            stg_q = work.tile([128, 128], f32, tag="stg")
            nc.vector.tensor_copy(
                stg_q[:].rearrange("p (b n) -> p b n", n=32),
                win32(hA, 240 * b4))
            pq_ = ps.tile([128, 512], f32, tag="ps")
            nc.tensor.transpose(pq_[0:128, 0:128], stg_q[:], ident[:])
            nc.scalar.activation(qnm_s[:, D * b4:D * (b4 + 1)], pq_[0:128, 0:128],
                                 AF.Copy)

        # moved = tp @ c_emb per pair; score accumulation
        sd_s = fin.tile([NC, BP], f32, tag="sd")
        for p in range(BP):
            b4, pi = p // 4, p % 4
            pm = ps.tile([128, 512], f32, tag="ps")
            nc.tensor.matmul(
                pm[0:NC, 0:D],
                lhsT=r(tpT_s[32 * pi:32 * pi + NC, NC * b4:NC * b4 + NC]),
                rhs=r(cnm_s[32 * pi:32 * pi + NC, D * b4:D * (b4 + 1)]),
                start=True, stop=True, tile_position=(32 * pi, 0))
            dif_s = work.tile([NC, D], f32, tag="dif")
            nc.vector.tensor_sub(dif_s[:], qnm_s[32 * pi:32 * pi + NC,
                                                 D * b4:D * (b4 + 1)], pm[0:NC, 0:D])
            nc.vector.tensor_relu(dif_s[:], dif_s[:])
            nc.vector.tensor_reduce(sd_s[:, p:p + 1], dif_s[:], axis=AX.X, op=ALU.add)
        pscore = ps.tile([128, 512], f32, tag="ps")
        nc.tensor.matmul(pscore[0:1, 0:BP], lhsT=r(ones_col[:]), rhs=r(sd_s[:]),
                         start=True, stop=True)
        score_row = work.tile([1, BP], f32, tag="srow")
        nc.scalar.activation(score_row[:], pscore[0:1, 0:BP], AF.Copy, scale=-1.0)
        nc.sync.dma_start(scores_out[:], score_row[0:1, :])

        work_cm.__exit__(None, None, None)
        fin_cm.__exit__(None, None, None)
        ps_cm.__exit__(None, None, None)
        persist_cm.__exit__(None, None, None)

    nc.compile()
    return nc


def _get_program(c_uniform, NGU, NS):
    key = (c_uniform, NGU, NS)
    if key not in _CACHE:
        _CACHE[key] = _build(c_uniform, NGU, NS)
    return _CACHE[key]


def kernel(**inputs) -> np.ndarray:
    from concourse.bass_utils import run_bass_kernel_spmd
    in_maps, c_uniform, NGU, NS = _host_prep(inputs)
    nc = _get_program(c_uniform, NGU, NS)
    res = run_bass_kernel_spmd(nc, in_maps, core_ids=list(range(NCORES)))
    scores = np.concatenate([res.results[c]['scores'] for c in range(NCORES)])
    return scores.astype(np.float32)



# revision 29
# speedup vs baseline: 2.4190x; 2.4190x over previous
"""Trainium2 Bass kernel for nn_AddingToQ (GNN message passing + sinkhorn).

Self-contained: takes FULL unsharded inputs, shards 256 graph pairs across
8 NeuronCores (32 pairs / 1920 nodes / 4320 real edges per core), runs an
all-SBUF matmul-formulated GNN, gathers per-core scores to the full [256]
output.

v2 restructuring (validated in numpy against the jax reference):
  * all-f32r propagation (fp32 bits, 1-pass PE rate at free>=256) vs the
    old fp32 node MLPs (4 cycles/row).
  * node tiles = 2 pairs compacted to 120 rows; per-block edge tiles (2 full
    128-tiles + packed remainder) -> every full edge tile gathers from
    exactly one node tile (48 incidences/direction vs 62).
  * msg-MLP edge constant c1 folded into row 120 of the U/V tiles: the
    gather one-hots carry an extra ones-row, so c1 costs zero instructions.
  * sinkhorn runs in the linear domain (row-max exp once, then 10 rounds of
    row/col divide) -> no exp/ln alternation, 2 act-table loads total.
  * final stage stays true-fp32 (precision: the 1/temp=10x logit scale
    amplifies any rounding into the transport plan).
"""
import numpy as np

# problem constants
B, NQ, NC = 256, 15, 30
NPG = 2 * NC
N = B * NPG
EPP = 135                 # real (mask=1) edges per pair
E_REAL = B * EPP
D, H, T = 128, 256, 64
N_PROP, SK_ITERS, SK_TEMP = 5, 10, 0.1
NCORES = 8
BP = B // NCORES          # 32 pairs per core
NL = BP * NPG             # 1920 nodes per core
EL = BP * EPP             # 4320 edges per core
NBLK = BP // 2            # 16 blocks (2 pairs = 120 nodes, 270 edges)
NFT = 32                  # full edge tiles (2 per block)
NRT = 2                   # remainder tiles (8 blocks x 14 edges = 112)
ET = NFT + NRT
NGU = NFT + 16            # gather incidences per direction
GS_COLS = NFT * 120 + 4 * 480   # scatter one-hot columns

_CACHE = {}


def _host_prep(inputs):
    f32 = np.float32
    msg_w1 = np.asarray(inputs['msg_w1'], f32)
    W1a, W1b, W1c = msg_w1[0:128], msg_w1[128:256], msg_w1[256:384]
    upd_w1 = np.asarray(inputs['upd_w1'], f32)
    A1, A2 = upd_w1[0:128], upd_w1[128:256]
    msg_w2 = np.asarray(inputs['msg_w2'], f32)
    M1 = (msg_w2 @ A1).astype(f32)
    b2A1 = (np.asarray(inputs['msg_b2'], f32) @ A1).astype(f32)
    upd_b1 = np.asarray(inputs['upd_b1'], f32)
    upd_w2 = np.asarray(inputs['upd_w2'], f32)
    upd_b2 = np.asarray(inputs['upd_b2'], f32)

    nf = np.asarray(inputs['node_features'], f32)
    h0 = nf * np.asarray(inputs['enc_node_w'], f32)[0][None, :] \
        + np.asarray(inputs['enc_node_b'], f32)[None, :]
    ef = np.asarray(inputs['edge_features'], f32)
    e_enc = ef * np.asarray(inputs['enc_edge_w'], f32)[0][None, :] \
        + np.asarray(inputs['enc_edge_b'], f32)[None, :]
    C_all = (e_enc @ W1c + np.asarray(inputs['msg_b1'], f32)[None, :]).astype(f32)
    assert bool(np.all(C_all[:E_REAL] == C_all[0])), "edge encodings not uniform"
    c1h = 0.5 * C_all[0]

    from_idx = np.asarray(inputs['from_idx']).astype(np.int64)
    to_idx = np.asarray(inputs['to_idx']).astype(np.int64)
    mask = np.asarray(inputs['mask_from_idx'], f32)
    assert np.all(mask[:E_REAL] == 1.0) and np.all(mask[E_REAL:] == 0.0)
    pair_of_edge = np.arange(E_REAL) // EPP
    assert np.all(from_idx[:E_REAL] // NPG == pair_of_edge)
    assert np.all(to_idx[:E_REAL] // NPG == pair_of_edge)

    # weights in exact SBUF layouts (same for all cores)
    w1ab = np.concatenate([W1a, W1b], axis=1)                     # [128, 512]
    m1 = np.concatenate([M1[0:128], M1[128:256]], axis=1)         # [128, 512]
    wu2 = np.concatenate([upd_w2[0:128], upd_w2[128:256]], axis=1)  # [128,256]
    updb1 = np.stack([upd_b1[0:128], upd_b1[128:256]], axis=1)    # [128, 2]
    c1pad = np.zeros((8, 16 * 512), f32)    # UV rows 120..127 (row 0 = c1/2)
    for k in range(16):
        c1pad[0, 512*k:512*k+256] = c1h
        c1pad[0, 512*k+256:512*k+512] = c1h
    # sinkhorn column-sum-broadcast ones (with junk-col fix) and score ones
    onesbd = np.zeros((128, 128), f32)
    onesq = np.zeros((128, 4), f32)
    for j in range(4):
        # junk cols (s>=30) get the same pattern: block colsums are positive,
        # so junk rows stay finite across iterations
        for s in range(32):
            onesbd[32*j:32*j+30, 32*j+s] = 1.0
        onesq[32*j:32*j+30, j] = 1.0

    common = {
        'w1ab': np.ascontiguousarray(w1ab), 'm1': np.ascontiguousarray(m1),
        'a2': np.ascontiguousarray(A2), 'wu2': np.ascontiguousarray(wu2),
        'b2a1': np.ascontiguousarray(b2A1[None, :]),
        'ub2': np.ascontiguousarray(upd_b2[None, :]),
        'updb1': np.ascontiguousarray(updb1),
        'c1pad': c1pad,
        'ft1': np.ascontiguousarray(np.asarray(inputs['ft1_w'], f32)),
        'ft2': np.ascontiguousarray(np.asarray(inputs['ft2_w'], f32)),
        'ft1b': np.ascontiguousarray(np.asarray(inputs['ft1_b'], f32)[:, None]),
        'ft2b': np.ascontiguousarray(np.asarray(inputs['ft2_b'], f32)[:, None]),
        'onesbd': onesbd, 'onesq': onesq,
    }

    in_maps = []
    for c in range(NCORES):
        n0, e0 = c * NL, c * EL
        fl = from_idx[e0:e0 + EL] - n0
        tl = to_idx[e0:e0 + EL] - n0
        assert fl.min() >= 0 and fl.max() < NL and tl.min() >= 0 and tl.max() < NL

        gu = np.zeros((128, NGU * 128), f32)
        gv = np.zeros((128, NGU * 128), f32)
        gs = np.zeros((128, GS_COLS), f32)
        for t in range(NFT):
            b, i = t // 2, t % 2
            es = slice(270*b + 128*i, 270*b + 128*i + 128)
            flb, tlb = fl[es] - 120*b, tl[es] - 120*b
            cols = np.arange(128)
            gu[flb, t*128 + cols] = 1.0
            gv[tlb, t*128 + cols] = 1.0
            gu[120, t*128:(t+1)*128] = 1.0
            gv[120, t*128:(t+1)*128] = 1.0
            gs[cols, t*120 + tlb] = 1.0
        for rt in range(NRT):
            for kk in range(8):
                bb = 8*rt + kk
                idx = NFT + 8*rt + kk
                js = 14*kk + np.arange(14)
                es = 270*bb + 256 + np.arange(14)
                flb, tlb = fl[es] - 120*bb, tl[es] - 120*bb
                gu[flb, idx*128 + js] = 1.0
                gv[tlb, idx*128 + js] = 1.0
                gu[120, idx*128 + js] = 1.0
                gv[120, idx*128 + js] = 1.0
                gg = bb // 4             # target group
                gcol = NFT*120 + (2*rt + (gg % 2)) * 480
                gs[js, gcol + 120*(bb % 4) + tlb] = 1.0

        indeg = np.zeros(NL, f32)
        np.add.at(indeg, tl, 1.0)

        m = {
            'ht0': np.ascontiguousarray(h0[n0:n0 + NL].T),        # [128,1920]
            'gu': gu, 'gv': gv, 'gs': gs,
            'indeg': np.ascontiguousarray(indeg[None, :]),
        }
        m.update(common)
        in_maps.append(m)
    return in_maps


def _build():
    """Build + schedule the Bass/Tile program (identical for all cores)."""
    import concourse.bass as bass
    import concourse.tile as tile
    from concourse import bacc, mybir
    from concourse.masks import make_identity

    f32 = mybir.dt.float32
    f32r = mybir.dt.float32r
    AF = mybir.ActivationFunctionType
    ALU = mybir.AluOpType
    AX = mybir.AxisListType

    nc = bacc.Bacc("TRN2", target_bir_lowering=False, debug=False)

    dram = {}
    def din(name, shape, dt_=f32):
        dram[name] = nc.dram_tensor(name, list(shape), dt_,
                                    kind="ExternalInput").ap()
    din('ht0', (128, NL))
    din('gu', (128, NGU * 128), f32r)
    din('gv', (128, NGU * 128), f32r)
    din('gs', (128, GS_COLS), f32r)
    din('indeg', (1, NL), f32r)
    din('w1ab', (128, 512), f32r); din('m1', (128, 512), f32r)
    din('a2', (128, H), f32r); din('wu2', (128, H), f32r)
    din('b2a1', (1, H), f32r); din('ub2', (1, 128), f32r)
    din('updb1', (128, 2))
    din('c1pad', (8, 16 * 512), f32r)
    din('ft1', (128, T)); din('ft2', (T, T))
    din('ft1b', (T, 1)); din('ft2b', (T, 1))
    din('onesbd', (128, 128)); din('onesq', (128, 4))
    scores_out = nc.dram_tensor('scores', [4, 8], f32, kind="ExternalOutput").ap()
    import os
    DBG = bool(os.environ.get('KERNEL_DEBUG'))
    if DBG:
        dbg_h = nc.dram_tensor('dbg_h', [128, NL], f32, kind="ExternalOutput").ap()
        dbg_al0 = nc.dram_tensor('dbg_al0', [128, 240], f32, kind="ExternalOutput").ap()
        dbg_al = nc.dram_tensor('dbg_al', [128, 240], f32, kind="ExternalOutput").ap()
        dbg_uv = nc.dram_tensor('dbg_uv', [128, 8192], mybir.dt.float32r, kind="ExternalOutput").ap()
        dbg_agg = nc.dram_tensor('dbg_agg', [128, 960], mybir.dt.float32r, kind="ExternalOutput").ap()
        dbg_rs = nc.dram_tensor('dbg_rs', [128, 8], f32, kind="ExternalOutput").ap()
        dbg_rr = nc.dram_tensor('dbg_rr', [128, 8], f32, kind="ExternalOutput").ap()
        dbg_alr = nc.dram_tensor('dbg_alr', [128, 240], f32, kind="ExternalOutput").ap()
        dbg_csb = nc.dram_tensor('dbg_csb', [128, 240], f32, kind="ExternalOutput").ap()
        dbg_crb = nc.dram_tensor('dbg_crb', [128, 240], f32, kind="ExternalOutput").ap()

    with tile.TileContext(nc) as tc:
        persist_cm = tc.tile_pool(name="persist", bufs=1)
        persist = persist_cm.__enter__()
        ps_cm = tc.tile_pool(name="ps", bufs=8, space="PSUM")
        ps = ps_cm.__enter__()

        def load(pool, name, shape, dt_=f32):
            t_ = pool.tile(list(shape), dt_, tag=name)
            nc.sync.dma_start(t_[:], dram[name][:])
            return t_

        # ---- persistent tensors ----
        # 32 pad cols so 60-strided win32 views in the final stage stay
        # in-bounds for the last pair
        hA = persist.tile([128, NL + 32], f32, tag="hA")
        nc.sync.dma_start(hA[:, 0:NL], dram['ht0'][:])
        nc.vector.memset(hA[:, NL:NL + 32], 0.0)
        # f32r shadow of h for matmul operands (f32r consumers require
        # producers that round; engine dtype-converting copies do)
        hr = persist.tile([128, NL], f32r, tag="hr")
        for g in range(4):
            nc.scalar.activation(hr[:, 480*g:480*g+480],
                                 hA[:, 480*g:480*g+480], AF.Copy)
        w1ab_s = load(persist, 'w1ab', (128, 512), f32r)
        m1_s = load(persist, 'm1', (128, 512), f32r)
        a2_s = load(persist, 'a2', (128, H), f32r)
        wu2_s = load(persist, 'wu2', (128, H), f32r)
        b2a1_s = load(persist, 'b2a1', (1, H), f32r)
        ub2_s = load(persist, 'ub2', (1, 128), f32r)
        updb1_s = load(persist, 'updb1', (128, 2))
        indeg_s = load(persist, 'indeg', (1, NL), f32r)
        ft1_s = load(persist, 'ft1', (128, T)); ft2_s = load(persist, 'ft2', (T, T))
        ft1b_s = load(persist, 'ft1b', (T, 1)); ft2b_s = load(persist, 'ft2b', (T, 1))
        onesbd_s = load(persist, 'onesbd', (128, 128))
        onesq_s = load(persist, 'onesq', (128, 4))
        ones_f = persist.tile([1, 512], f32, tag="ones_f")
        nc.vector.memset(ones_f[:], 1.0)
        ones_r = persist.tile([1, 512], f32r, tag="ones_r")
        nc.scalar.activation(ones_r[:], ones_f[:], AF.Copy)
        ident = persist.tile([128, 128], f32, tag="ident")
        make_identity(nc, ident[:])

        # ---- propagation-scoped pools ----
        mask_cm = tc.tile_pool(name="maskp", bufs=1)
        maskp = mask_cm.__enter__()
        uv_cm = tc.tile_pool(name="uvp", bufs=1)
        uvp = uv_cm.__enter__()
        agg_cm = tc.tile_pool(name="aggp", bufs=2)
        aggpool = agg_cm.__enter__()
        rg_cm = tc.tile_pool(name="rgp", bufs=2)
        rgpool = rg_cm.__enter__()
        relu_cm = tc.tile_pool(name="relu1", bufs=36)
        relu_pool = relu_cm.__enter__()

        # chunked mask DMA so layer-0 gathers can start early
        gu_a = maskp.tile([128, 24 * 128], f32r, tag="gu_a")
        gu_b = maskp.tile([128, 24 * 128], f32r, tag="gu_b")
        gv_a = maskp.tile([128, 24 * 128], f32r, tag="gv_a")
        gv_b = maskp.tile([128, 24 * 128], f32r, tag="gv_b")
        nc.sync.dma_start(gu_a[:], dram['gu'][:, 0:3072])
        nc.sync.dma_start(gv_a[:], dram['gv'][:, 0:3072])
        nc.sync.dma_start(gu_b[:], dram['gu'][:, 3072:6144])
        nc.sync.dma_start(gv_b[:], dram['gv'][:, 3072:6144])
        gs_s = maskp.tile([128, GS_COLS], f32r, tag="gs")
        nc.sync.dma_start(gs_s[:], dram['gs'][:])

        def gu_ap(idx):
            return (gu_a if idx < 24 else gu_b)[:, (idx % 24)*128:(idx % 24)*128+128]

        def gv_ap(idx):
            return (gv_a if idx < 24 else gv_b)[:, (idx % 24)*128:(idx % 24)*128+128]

        UV_s = uvp.tile([128, 16 * 512], f32r, tag="UV")
        # rows 120..127: row 120 = c1/2 constants, 121..127 zeros (stage A
        # rewrites rows 0:120 every layer before the gathers read them)
        nc.sync.dma_start(UV_s[120:128, :], dram['c1pad'][:])

        # per-tile gather incidence lists: (uv_tile_k, gu_col_idx)
        gath = {}
        for t in range(NFT):
            gath[t] = [(t // 2, t)]
        for rt in range(NRT):
            gath[NFT + rt] = [(8*rt + kk, NFT + 8*rt + kk) for kk in range(8)]

        for layer in range(N_PROP):
            # --- stage A: UV[k] = h_k @ [W1a|W1b] (rows 0:120) ---
            for k in range(16):
                pu = ps.tile([128, 512], f32, tag="ps")
                nc.tensor.matmul(pu[0:120, 0:512],
                                 lhsT=hr[:, 120*k:120*k+120],
                                 rhs=w1ab_s[:], start=True, stop=True)
                if k % 2 == 0:
                    nc.scalar.activation(UV_s[0:120, 512*k:512*(k+1)],
                                         pu[0:120, 0:512], AF.Copy)
                else:
                    nc.vector.tensor_copy(UV_s[0:120, 512*k:512*(k+1)],
                                          pu[0:120, 0:512])

            # --- gathers + relu (remainder tiles first: groups need them) ---
            relu_t = {}
            order = [NFT, NFT + 1] + list(range(NFT))
            for t in order:
                inc = gath[t]
                pp = ps.tile([128, 512], f32, tag="ps")
                for j, (k, idx) in enumerate(inc):
                    nc.tensor.matmul(pp[:, 0:256], lhsT=gu_ap(idx),
                                     rhs=UV_s[:, 512*k:512*k+256],
                                     start=(j == 0), stop=False)
                    nc.tensor.matmul(pp[:, 0:256], lhsT=gv_ap(idx),
                                     rhs=UV_s[:, 512*k+256:512*k+512],
                                     start=False, stop=(j == len(inc) - 1))
                rt_ = relu_pool.tile([128, 256], f32r, tag="r1")
                nc.vector.tensor_relu(rt_[:], pp[:, 0:256])
                relu_t[t] = rt_

            # --- per 480-node group: scatter + update ---
            for g in range(4):
                agg_h0 = ps.tile([128, 512], f32, tag="ps")
                agg_h1 = ps.tile([128, 512], f32, tag="ps")
                aggp = [agg_h0, agg_h1]
                rt_idx = NFT + (0 if g < 2 else 1)
                rcol = NFT*120 + (2*(rt_idx - NFT) + (g % 2)) * 480
                for hh in range(2):
                    nc.tensor.matmul(aggp[hh][:, 0:480],
                                     lhsT=relu_t[rt_idx][:, 128*hh:128*hh+128],
                                     rhs=gs_s[:, rcol:rcol+480],
                                     start=True, stop=False)
                for bi in range(4):
                    b = 4*g + bi
                    for i in range(2):
                        t = 2*b + i
                        last = (bi == 3 and i == 1)
                        for hh in range(2):
                            nc.tensor.matmul(
                                aggp[hh][:, 120*bi:120*bi+120],
                                lhsT=relu_t[t][:, 128*hh:128*hh+128],
                                rhs=gs_s[:, t*120:t*120+120],
                                start=False, stop=last, skip_group_check=True)
                agg_s = aggpool.tile([128, 960], f32r, tag="agg")
                nc.scalar.activation(agg_s[:, 0:480], aggp[0][:, 0:480], AF.Copy)
                nc.vector.tensor_copy(agg_s[:, 480:960], aggp[1][:, 0:480])
                if DBG and layer == 0 and g == 0:
                    nc.sync.dma_start(dbg_agg[:], agg_s[:])

                ns = slice(480*g, 480*g+480)
                rg_s = rgpool.tile([128, 960], f32r, tag="rg")
                for hh in range(2):
                    pq = ps.tile([128, 512], f32, tag="ps")
                    nc.tensor.matmul(pq[:, 0:480], lhsT=m1_s[:, 128*hh:128*hh+128],
                                     rhs=agg_s[:, 0:480], start=True, stop=False)
                    nc.tensor.matmul(pq[:, 0:480],
                                     lhsT=m1_s[:, 256+128*hh:256+128*hh+128],
                                     rhs=agg_s[:, 480:960], start=False, stop=False)
                    nc.tensor.matmul(pq[:, 0:480], lhsT=a2_s[:, 128*hh:128*hh+128],
                                     rhs=hr[:, ns],
                                     start=False, stop=False)
                    nc.tensor.matmul(pq[:, 0:480], lhsT=b2a1_s[0:1, 128*hh:128*hh+128],
                                     rhs=indeg_s[0:1, ns], start=False, stop=True)
                    nc.scalar.activation(rg_s[:, 480*hh:480*hh+480], pq[:, 0:480],
                                         AF.Relu, bias=updb1_s[:, hh:hh+1])
                pd = ps.tile([128, 512], f32, tag="ps")
                nc.tensor.matmul(pd[:, 0:480], lhsT=wu2_s[:, 0:128],
                                 rhs=rg_s[:, 0:480], start=True, stop=False)
                nc.tensor.matmul(pd[:, 0:480], lhsT=wu2_s[:, 128:256],
                                 rhs=rg_s[:, 480:960], start=False, stop=False)
                nc.tensor.matmul(pd[:, 0:480], lhsT=ub2_s[:],
                                 rhs=ones_r[0:1, 0:480], start=False, stop=True)
                nc.vector.tensor_add(hA[:, ns], hA[:, ns], pd[:, 0:480])
                if layer < N_PROP - 1:
                    nc.scalar.activation(hr[:, ns], hA[:, ns], AF.Copy)

        if DBG:
            nc.sync.dma_start(dbg_h[:], hA[:, 0:NL])
            nc.sync.dma_start(dbg_uv[:], UV_s[:])
        # close propagation pools
        relu_cm.__exit__(None, None, None)
        rg_cm.__exit__(None, None, None)
        agg_cm.__exit__(None, None, None)
        uv_cm.__exit__(None, None, None)
        mask_cm.__exit__(None, None, None)

        fin_cm = tc.tile_pool(name="fin", bufs=1)
        fin = fin_cm.__enter__()
        work_cm = tc.tile_pool(name="work", bufs=4)
        work = work_cm.__enter__()

        # ---- final stage (fp32) ----
        # transforms: s1 = relu(ft1^T h + b1); tT = ft2^T s1 + b2
        s1_s = fin.tile([T, NL], f32, tag="s1")
        tT_s = fin.tile([T, NL], f32, tag="tT")
        for j in range(4):
            cs = slice(480*j, 480*(j+1))
            p1 = ps.tile([128, 512], f32, tag="ps")
            nc.tensor.matmul(p1[0:T, 0:480], lhsT=ft1_s[:], rhs=hA[:, cs],
                             start=True, stop=True)
            nc.scalar.activation(s1_s[:, cs], p1[0:T, 0:480], AF.Relu, bias=ft1b_s[:])
            p2 = ps.tile([128, 512], f32, tag="ps")
            nc.tensor.matmul(p2[0:T, 0:480], lhsT=ft2_s[:], rhs=s1_s[:, cs],
                             start=True, stop=True)
            nc.scalar.activation(tT_s[:, cs], p2[0:T, 0:480], AF.Identity,
                                 bias=ft2b_s[:])

        # masked query transform: mtq [T, BP*NC], zero at q>=NQ
        mtq_s = fin.tile([T, BP * NC], f32, tag="mtq")
        nc.vector.memset(mtq_s[:], 0.0)
        nc.vector.tensor_copy(
            mtq_s[:].rearrange("p (b n) -> p b n", n=NC)[:, :, 0:NQ],
            tT_s[:].rearrange("p (b n) -> p b n", n=NPG)[:, :, 0:NQ])

        # log-alpha: pair p=(j=p%4 row-block, g=p//4 col-group) -> [128, 240]
        pla = ps.tile([128, 512], f32, tag="ps")
        for p in range(BP):
            j, g = p % 4, p // 4
            nc.tensor.matmul(pla[32*j:32*j+30, 30*g:30*g+30],
                             lhsT=mtq_s[0:T, 30*p:30*p+30],
                             rhs=tT_s[0:T, NPG*p+NC:NPG*p+2*NC],
                             start=True, stop=True, tile_position=(0, 32*j))
        # row-max subtract (in psum), then exp(10*x) into alpha
        al_s = fin.tile([128, 240], f32, tag="al")
        nc.vector.memset(al_s[:], 1.0)
        mx_s = work.tile([128, 8], f32, tag="mx")
        pla3 = pla[:, 0:240].rearrange("p (a b) -> p a b", b=NC)
        nc.vector.tensor_reduce(mx_s[:], pla3, axis=AX.X, op=ALU.max)
        nc.vector.tensor_tensor(pla3, pla3,
                                mx_s[:, :, None].broadcast_to([128, 8, NC]),
                                op=ALU.subtract)
        for j in range(4):
            nc.scalar.activation(al_s[32*j:32*j+30, :], pla[32*j:32*j+30, 0:240],
                                 AF.Exp, scale=1.0 / SK_TEMP)

        if DBG:
            nc.sync.dma_start(dbg_al0[:], al_s[:])
        # linear-domain sinkhorn
        al3 = al_s[:].rearrange("p (a b) -> p a b", b=NC)
        rs_s = work.tile([128, 8], f32, tag="rs")
        rr_s = work.tile([128, 8], f32, tag="rr")
        crb_s = fin.tile([128, 240], f32, tag="crb")
        csb_s = fin.tile([128, 240], f32, tag="csb")
        for it in range(SK_ITERS):
            nc.vector.tensor_reduce(rs_s[:], al3, axis=AX.X, op=ALU.add)
            nc.vector.reciprocal(rr_s[:], rs_s[:])
            nc.vector.tensor_tensor(al3, al3,
                                    rr_s[:, :, None].broadcast_to([128, 8, NC]),
                                    op=ALU.mult)
            pcb = ps.tile([128, 512], f32, tag="ps")
            nc.tensor.matmul(pcb[:, 0:240], lhsT=onesbd_s[:], rhs=al_s[:],
                             start=True, stop=True)
            nc.vector.tensor_copy(csb_s[:], pcb[:, 0:240])
            nc.vector.reciprocal_approx_fast(out=crb_s[:], in_=csb_s[:])
            if DBG and it == 0:
                nc.sync.dma_start(dbg_rs[:], rs_s[:])
                nc.sync.dma_start(dbg_rr[:], rr_s[:])
                nc.sync.dma_start(dbg_alr[:], al_s[:])
                nc.sync.dma_start(dbg_csb[:], csb_s[:])
                nc.sync.dma_start(dbg_crb[:], crb_s[:])
            nc.vector.tensor_tensor(al_s[:], al_s[:], crb_s[:], op=ALU.mult)

        if DBG:
            nc.sync.dma_start(dbg_al[:], al_s[:])
        # transport-plan transposes: per col-group g, [128,30] -> [30,128]
        # (c at base 0, q of pair (j,g) on free cols 32j..32j+29)
        tpT_s = fin.tile([30, 8 * 128], f32, tag="tpT")
        for g in range(8):
            ptp = ps.tile([128, 512], f32, tag="ps")
            nc.tensor.transpose(ptp[0:30, 0:128], al_s[:, 30*g:30*g+30], ident[:])
            nc.vector.tensor_copy(tpT_s[:, 128*g:128*(g+1)], ptp[0:30, 0:128])

        # c embeddings per pair, c-major [30, 128], straight from hA
        cnm_s = fin.tile([30, BP * D], f32, tag="cnm")
        for p in range(BP):
            pc_ = ps.tile([128, 512], f32, tag="ps")
            nc.tensor.transpose(pc_[0:30, 0:128], hA[:, NPG*p+NC:NPG*p+2*NC],
                                ident[:])
            if p % 2 == 0:
                nc.scalar.activation(cnm_s[:, D*p:D*(p+1)], pc_[0:30, 0:128],
                                     AF.Copy)
            else:
                nc.vector.tensor_copy(cnm_s[:, D*p:D*(p+1)], pc_[0:30, 0:128])

        # q embeddings node-major at 32-stride (4 pairs per 128-col slab)
        qnm_s = fin.tile([128, 8 * D], f32, tag="qnm")

        def win32(off):
            w = hA[:, off:off + 240]
            return w.rearrange("p (b n) -> p b n", n=NPG)[:, :, 0:32]

        for b4 in range(8):
            stg_q = work.tile([128, 128], f32, tag="stg")
            nc.vector.tensor_copy(
                stg_q[:].rearrange("p (b n) -> p b n", n=32), win32(240*b4))
            pq_ = ps.tile([128, 512], f32, tag="ps")
            nc.tensor.transpose(pq_[0:128, 0:128], stg_q[:], ident[:])
            nc.scalar.activation(qnm_s[:, D*b4:D*(b4+1)], pq_[0:128, 0:128], AF.Copy)

        # moved = tp @ c_emb (4 pairs batched per group psum), then scores
        sd_s = fin.tile([128, 8], f32, tag="sd")
        for g in range(8):
            pm = ps.tile([128, 512], f32, tag="ps")
            nc.vector.memset(pm[:, 0:128], 0.0)
            for j in range(4):
                p = 4*g + j
                nc.tensor.matmul(pm[32*j:32*j+30, 0:128],
                                 lhsT=tpT_s[0:30, 128*g+32*j:128*g+32*j+30],
                                 rhs=cnm_s[0:30, D*p:D*(p+1)],
                                 start=True, stop=True, tile_position=(0, 32*j))
            dif = work.tile([128, 128], f32, tag="dif")
            nc.vector.tensor_sub(dif[:], qnm_s[:, D*g:D*(g+1)], pm[:, 0:128])
            nc.scalar.activation(dif[:], dif[:], AF.Relu)
            nc.vector.tensor_reduce(sd_s[:, g:g+1], dif[:], axis=AX.X, op=ALU.add)
        psc = ps.tile([128, 512], f32, tag="ps")
        nc.tensor.matmul(psc[0:4, 0:8], lhsT=onesq_s[:], rhs=sd_s[:],
                         start=True, stop=True)
        score_row = work.tile([4, 8], f32, tag="srow")
        nc.scalar.activation(score_row[:], psc[0:4, 0:8], AF.Copy, scale=-1.0)
        nc.sync.dma_start(scores_out[:], score_row[:])

        work_cm.__exit__(None, None, None)
        fin_cm.__exit__(None, None, None)
        ps_cm.__exit__(None, None, None)
        persist_cm.__exit__(None, None, None)

    nc.compile()
    return nc


def _get_program():
    if 'nc' not in _CACHE:
        _CACHE['nc'] = _build()
    return _CACHE['nc']


def kernel(**inputs) -> np.ndarray:
    from concourse.bass_utils import run_bass_kernel_spmd
    in_maps = _host_prep(inputs)
    nc = _get_program()
    res = run_bass_kernel_spmd(nc, in_maps, core_ids=list(range(NCORES)))
    out = np.zeros(B, np.float32)
    for c in range(NCORES):
        r = np.asarray(res.results[c]['scores'])   # [4, 8]
        for p in range(BP):
            out[c*BP + p] = r[p % 4, p // 4]
    return out.astype(np.float32)
